# revision 2
# baseline (speedup 1.0000x reference)
"""BioRNN Trainium2 kernel v2: 8-core tensor-parallel recurrence.

Per step: AllGather bf16 rates -> 32 col-tiled matmuls (4 PSUM strips, one
bank) -> fold-transpose on PE -> lean epilogue -> ship rates. vs v1:
  - readout computed from the core's OWN rates during the AllGather window
    (w_out zeroed on cores != 0), off the post-gather critical path
  - strip copies: one N=256 ACT copy + one N=256 DVE copy in parallel;
    fold matmuls ordered to chase copy completion (c2,c3,c0,c1)
  - epilogue on-path is 5 full-width ops via
        ime' = ime*(1-alpha) + alpha*relu(pre)
        h'   = (base + ps2) + alpha*relu1,  base = 0.8h + noise' + ime*(1-a)
    with alpha/(1-alpha) materialized as [128,128] tensors (per-partition
    scalars cannot express per-chunk coefficients), base precomputed during
    the AllGather window; relu rates on DVE in parallel with sigmoid on ACT.
"""
import sys
sys.path.insert(0, '/opt/trn_rl_repo')
import numpy as np

import concourse.bass as bass
import concourse.mybir as mybir

try:
    import ml_dtypes
    BF16 = ml_dtypes.bfloat16
except ImportError:  # pragma: no cover
    import jax.numpy as jnp
    BF16 = jnp.bfloat16

# ---- model constants (hardcoded from the problem spec) ----
SIZES = [512, 1024, 128, 128, 128, 512, 1024, 128, 128, 128]
OFF = np.cumsum([0] + SIZES)
N = int(OFF[-1])            # 3840
NP_ = 4096                  # padded
N_BR = 2
N_IN, N_OUT = 128, 3
T_FULL, B = 100, 32
DECAY = np.float32(10.0 / 50.0)
NOISE_STD = 0.01
N_CORES = 8
SHARD = NP_ // N_CORES      # 512 neurons per core
NCH = NP_ // 128            # 32 k-chunks
CCH = SHARD // 128          # 4 chunks per core
FREE = CCH * B              # 128 free cols of state tiles

_tau_me = np.tile(np.logspace(np.log10(100.0), np.log10(5000.0), SIZES[6] // N_BR), N_BR)
ALPHA_ME = (10.0 / _tau_me).astype(np.float32)

DT32 = mybir.dt.float32
DTBF = mybir.dt.bfloat16
AF = mybir.ActivationFunctionType
ALU = mybir.AluOpType


def build_kernel(T=T_FULL, has_bias=False):
    import os
    variant = os.environ.get("BIO_VARIANT", "")
    rd_own = os.environ.get("BIO_RD", "own") == "own"
    cp_split = os.environ.get("BIO_CP", "act") == "split"
    epi_v2 = os.environ.get("BIO_EPI", "v2") == "v2"
    n_dummy = int(os.environ.get("BIO_DUMMY", "24"))
    rr_dve = os.environ.get("BIO_RR", "dve") == "dve"
    nc = bass.Bass("TRN2", num_devices=N_CORES)

    # ---- DRAM parameters (per-core shards prepped on host) ----
    w_d = nc.declare_dram_parameter("w", [128, NCH * SHARD], DTBF, isOutput=False)
    win_d = nc.declare_dram_parameter("win", [128, SHARD], DTBF, isOutput=False)
    xt_d = nc.declare_dram_parameter("xt", [128, T * B], DTBF, isOutput=False)
    noise_d = nc.declare_dram_parameter("noise", [128, T * FREE], DT32, isOutput=False)
    wout_d = nc.declare_dram_parameter("wout", [128, CCH * N_OUT], DTBF, isOutput=False)
    alpt_d = nc.declare_dram_parameter("alpt", [128, 2 * FREE], DT32, isOutput=False)
    dmask_d = nc.declare_dram_parameter("dmask", [128, FREE], mybir.dt.uint8, isOutput=False)
    fmat_d = nc.declare_dram_parameter("fmat", [128, B], DTBF, isOutput=False)
    bout_d = nc.declare_dram_parameter("bout", [N_OUT, 1], DT32, isOutput=False)
    if has_bias:
        bb_d = nc.declare_dram_parameter("bb", [1, SHARD + B], DTBF, isOutput=False)
    out_d = nc.declare_dram_parameter("out", [N_OUT, T * B], DT32, isOutput=True)

    # ---- collective bounce buffers ----
    in_b = [nc.dram_tensor(f"in_b{p}", [128, FREE], DTBF) for p in range(2)]
    out_b = [nc.dram_tensor(f"out_b{p}", [128 * N_CORES, FREE], DTBF, addr_space="Shared")
             for p in range(2)]

    from contextlib import ExitStack
    with ExitStack() as ctx:
        block = ctx.enter_context(nc.Block())
        sems = {n: ctx.enter_context(nc.semaphore(n)) for n in
                ["DINIT", "DO", "DI", "DI2", "DI3", "DI4", "CC",
                 "PEA", "PEF", "PEO", "ACA", "ACV", "AI", "AR", "PSF",
                 "VH", "VR", "VO", "DO2"]}
        DINIT = sems["DINIT"]; DO = sems["DO"]; CC = sems["CC"]; DO2 = sems["DO2"]
        DI = sems["DI"]; DI2 = sems["DI2"]; DI3 = sems["DI3"]; DI4 = sems["DI4"]
        PEA = sems["PEA"]; PEF = sems["PEF"]; PEO = sems["PEO"]
        ACA = sems["ACA"]; ACV = sems["ACV"]; AI = sems["AI"]; AR = sems["AR"]
        PSF = sems["PSF"]; VH = sems["VH"]; VR = sems["VR"]; VO = sems["VO"]

        def sb(name, shape, dt):
            return ctx.enter_context(nc.sbuf_tensor(name, shape, dt))

        w_sb = sb("w_sb", [128, NCH * SHARD], DTBF)
        win_sb = sb("win_sb", [128, SHARD], DTBF)
        xt_sb = sb("xt_sb", [128, T * B], DTBF)
        noise_sb = sb("noise_sb", [128, T * FREE], DT32)
        wout_sb = sb("wout_sb", [128, CCH * N_OUT], DTBF)
        alpt_sb = sb("alpt_sb", [128, 2 * FREE], DT32)
        dmask_sb = sb("dmask_sb", [128, FREE], mybir.dt.uint8)
        fmat_sb = sb("fmat_sb", [128, B], DTBF)
        bout_sb = sb("bout_sb", [N_OUT, 1], DT32)
        if has_bias:
            bb_sb = sb("bb_sb", [1, SHARD + B], DTBF)
        g_sb = sb("g_sb", [128, N_CORES * FREE], DTBF)
        s_sb = sb("s_sb", [128, SHARD], DTBF)
        h_sb = sb("h_sb", [128, FREE], DT32)
        hn_sb = sb("hn_sb", [128, FREE], DT32)
        imeb_sb = sb("imeb_sb", [128, FREE], DT32)
        base_sb = sb("base_sb", [128, FREE], DT32)
        h1_sb = sb("h1_sb", [128, FREE], DT32)
        ar1_sb = sb("ar1_sb", [128, FREE], DT32)
        ime_sb = sb("ime_sb", [128, FREE], DT32)
        rs_sb = sb("rs_sb", [128, FREE], DT32)
        rl_sb = sb("rl_sb", [128, FREE], DT32)
        rr_sb = sb("rr_sb", [128, FREE], DT32)
        r_sb = sb("r_sb", [128, FREE], DTBF)
        o_sb = sb("o_sb", [N_OUT, T * B], DT32)
        ps1 = ctx.enter_context(nc.psum_tensor("ps1", [128, SHARD], DT32))
        ps2 = ctx.enter_context(nc.psum_tensor("ps2", [128, FREE], DT32))
        ps3 = ctx.enter_context(nc.psum_tensor("ps3", [N_OUT, B], DT32))
        psD = (ctx.enter_context(nc.psum_tensor("psD", [128, SHARD], DT32))
               if n_dummy else None)

        init_pairs = [
            (w_sb, w_d), (win_sb, win_d), (xt_sb, xt_d), (noise_sb, noise_d),
            (wout_sb, wout_d), (alpt_sb, alpt_d), (dmask_sb, dmask_d),
            (fmat_sb, fmat_d), (bout_sb, bout_d),
        ]
        if has_bias:
            init_pairs.append((bb_sb, bb_d))
        N_INIT_DMA = len(init_pairs)

        @block.sync
        def _(sync):
            for dst, src in init_pairs:
                sync.dma_start(out=dst[:, :], in_=src[:, :]).then_inc(DINIT, 16)
            for t in range(T):
                p = t % 2
                # ship r_t lower half
                sync.dma_start(out=in_b[p][0:64, :], in_=r_sb[0:64, :]).wait_op(
                    VR, t + 1, "sem-ge").then_inc(DO, 16)
                if not rd_own and t > 0:
                    sync.wait_ge(PEO, t)
                # gather-in quarters 0-1 (ranks 0-3)
                for q, sem in [(0, DI), (1, DI2)]:
                    ob = out_b[p][256 * q:256 * (q + 1), :].rearrange("(c p) n -> p c n", p=128)
                    gb = g_sb[:, 256 * q:256 * (q + 1)].rearrange("p (c n) -> p c n", c=2)
                    sync.dma_start(out=gb, in_=ob).wait_op(CC, t + 1, "sem-ge").then_inc(sem, 16)
            sync.wait_ge(VO, T)
            sync.dma_start(out=out_d[:, :], in_=o_sb[:, :]).then_inc(DO2, 16)

        @block.gpsimd
        def _(gpsimd):
            for t in range(T):
                p = t % 2
                if variant == "noag":
                    gpsimd.wait_ge(DO, 32 * (t + 1))
                    gpsimd.sem_inc(CC, 1)
                else:
                    gpsimd.collective_compute(
                        "AllGather",
                        ALU.bypass,
                        replica_groups=[list(range(N_CORES))],
                        ins=[in_b[p].ap().opt()],
                        outs=[out_b[p].ap().opt()],
                    ).wait_op(DO, 32 * (t + 1), "sem-ge").then_inc(CC)

        @block.tensor
        def _(pe):
            pe.wait_ge(DINIT, 16 * N_INIT_DMA)
            for t in range(T):
                # ps1 free after copies of t-1
                if t > 0:
                    pe.wait_ge(ACA, t)
                    pe.wait_ge(ACV, t)
                # x_t into strip 0 (runs during the AllGather)
                nc.tensor.matmul(
                    out=ps1[0:32, :],
                    lhsT=xt_sb[:, B * t:B * (t + 1)],
                    rhs=win_sb[:, :],
                    start=True, stop=False,
                    tile_position=(0, 0),
                    skip_group_check=True,
                )
                # readout from own rates r_t (during the AllGather)
                if rd_own:
                    pe.wait_ge(VR, t + 1)
                    if t > 0:
                        pe.wait_ge(VO, t)  # ps3 free
                    for c in range(CCH):
                        mm = nc.tensor.matmul(
                            out=ps3[:, :],
                            lhsT=wout_sb[:, N_OUT * c:N_OUT * (c + 1)],
                            rhs=r_sb[:, B * c:B * (c + 1)],
                            start=(c == 0), stop=(c == CCH - 1),
                            skip_group_check=True,
                        )
                    mm.then_inc(PEO, 1)
                # dummy matmuls to keep PE busy through the AllGather window
                for dmy in range(n_dummy):
                    nc.tensor.matmul(
                        out=psD[0:32, :],
                        lhsT=xt_sb[:, 0:32],
                        rhs=w_sb[:, 0:SHARD],
                        start=True, stop=True,
                        skip_group_check=True,
                    )
                # main recurrent matmuls: 8 groups x 4 col-tiled strips
                pe.wait_ge(DI, 16 * (t + 1))
                waited = {0}
                for g in range(0 if variant == "nomm" else 8):
                    q = g // 2
                    if q not in waited:
                        pe.wait_ge([DI, DI2, DI3, DI4][q], 16 * (t + 1))
                        waited.add(q)
                    for j in range(4):
                        kc = 4 * g + j
                        mm = nc.tensor.matmul(
                            out=ps1[32 * j:32 * (j + 1), :],
                            lhsT=g_sb[:, 32 * kc:32 * (kc + 1)],
                            rhs=w_sb[:, SHARD * kc:SHARD * (kc + 1)],
                            start=(g == 0 and j != 0),
                            stop=(g == 7),
                            skip_group_check=True,
                            tile_position=(0, 32 * j),
                        )
                if variant == "nomm":
                    mm = nc.tensor.matmul(
                        out=ps1[0:32, 0:32], lhsT=xt_sb[:, 0:32],
                        rhs=win_sb[:, 0:32], start=False, stop=True,
                        skip_group_check=True)
                mm.then_inc(PEA, 1)
                # ps2 group: optional mGluR bias, then fold-transpose chasing copies
                if t > 0:
                    pe.wait_ge(AI, (t if epi_v2 else CCH * t))
                    pe.wait_ge(PSF, t)
                if has_bias:
                    for c in range(CCH):
                        nc.tensor.matmul(
                            out=ps2[:, B * c:B * (c + 1)],
                            lhsT=bb_sb[0:1, 128 * c:128 * (c + 1)],
                            rhs=bb_sb[0:1, SHARD:SHARD + B],
                            start=(c == 0), stop=False,
                            skip_group_check=True,
                        )
                for i, c in enumerate((2, 3, 0, 1)):
                    pe.wait_ge(ACV if c >= 2 else ACA, t + 1)
                    mm = nc.tensor.matmul(
                        out=ps2[:, B * c:B * (c + 1)],
                        lhsT=s_sb[:, 128 * c:128 * (c + 1)],
                        rhs=fmat_sb[:, :],
                        start=(i == 0 and not has_bias), stop=(i == 3),
                        skip_group_check=True,
                    )
                mm.then_inc(PEF, 1)
                if not rd_own:
                    if t > 0:
                        pe.wait_ge(VO, t)  # ps3 free
                    for c in range(CCH):
                        mm = nc.tensor.matmul(
                            out=ps3[:, :],
                            lhsT=wout_sb[:, N_OUT * c:N_OUT * (c + 1)],
                            rhs=g_sb[:, B * c:B * (c + 1)],
                            start=(c == 0), stop=(c == CCH - 1),
                            skip_group_check=True,
                        )
                    mm.then_inc(PEO, 1)

        @block.scalar
        def _(act):
            act.wait_ge(DINIT, 16 * N_INIT_DMA)
            # r_0 sigmoid half from h_0 = 0
            act.wait_ge(VH, 1)
            nc.scalar.activation(rs_sb[:, :], h_sb[:, :], AF.Sigmoid).then_inc(AR, 1)
            for t in range(T):
                p = t % 2
                # ship r_t upper half + gather-in quarters 2-3 (DMA queue ops)
                act.dma_start(out=in_b[p][64:128, :], in_=r_sb[64:128, :]).wait_op(
                    VR, t + 1, "sem-ge").then_inc(DO, 16)
                for q, sem in [(2, DI3), (3, DI4)]:
                    ob2 = out_b[p][256 * q:256 * (q + 1), :].rearrange("(c p) n -> p c n", p=128)
                    gb2 = g_sb[:, 256 * q:256 * (q + 1)].rearrange("p (c n) -> p c n", c=2)
                    act.dma_start(out=gb2, in_=ob2).wait_op(CC, t + 1, "sem-ge").then_inc(sem, 16)
                # strips chunks 0-1 -> s_sb (one N=256 copy)
                act.wait_ge(PEA, t + 1)
                if not cp_split:
                    nc.scalar.copy(out=s_sb[:, 256:512], in_=ps1[:, 256:512]).then_inc(ACV, 1)
                nc.scalar.copy(out=s_sb[:, 0:256], in_=ps1[:, 0:256]).then_inc(ACA, 1)
                if epi_v2:
                    # relu1 = relu(ps2)  (= DECAY * relu(pre))
                    act.wait_ge(PEF, t + 1)
                    nc.scalar.activation(rl_sb[:, :], ps2[:, :], AF.Relu).then_inc(AI, 1)
                    # sigmoid rates from h_{t+1}
                    act.wait_ge(VH, t + 2)
                    if rr_dve:
                        nc.scalar.activation(rs_sb[:, :], h_sb[:, :], AF.Sigmoid).then_inc(AR, 1)
                    else:
                        nc.scalar.activation(rs_sb[:, :], h_sb[:, :], AF.Sigmoid)
                        nc.scalar.activation(rr_sb[:, :], h_sb[:, :], AF.Relu).then_inc(AR, 1)
                else:
                    act.wait_ge(PEF, t + 1)
                    for c in range(CCH):
                        nc.scalar.activation(
                            rl_sb[:, B * c:B * (c + 1)],
                            ps2[:, B * c:B * (c + 1)],
                            AF.Relu,
                            scale=alpt_sb[:, B * c:B * c + 1],
                        ).then_inc(AI, 1)
                    act.wait_ge(VH, t + 2)
                    nc.scalar.activation(rs_sb[:, :], h_sb[:, :], AF.Sigmoid)
                    nc.scalar.activation(rr_sb[:, :], h_sb[:, :], AF.Relu).then_inc(AR, 1)

        @block.vector
        def _(dve):
            dve.wait_ge(DINIT, 16 * N_INIT_DMA)
            dve.memset(h_sb[:, :], 0.0)
            dve.memset(ime_sb[:, :], 0.0)
            dve.memset(rr_sb[:, :], 0.0).then_inc(VH, 1)
            dve.drain()
            dve.wait_ge(AR, 1)
            nc.vector.select(r_sb[:, :], dmask_sb[:, :], rs_sb[:, :], rr_sb[:, :],
                             add_drain=True).then_inc(VR, 1)
            for t in range(T):
                # off-path during the AllGather:
                # hn = 0.8h + noise' ; imeb = ime*(1-alpha) ; base = hn + imeb
                nc.vector.scalar_tensor_tensor(
                    out=hn_sb[:, :], in0=h_sb[:, :], scalar=float(1.0 - DECAY),
                    in1=noise_sb[:, FREE * t:FREE * (t + 1)], op0=ALU.mult, op1=ALU.add,
                )
                if epi_v2:
                    nc.vector.tensor_tensor(
                        out=imeb_sb[:, :], in0=ime_sb[:, :], in1=alpt_sb[:, FREE:2 * FREE],
                        op=ALU.mult)
                    dve.drain()
                    nc.vector.tensor_tensor(
                        out=base_sb[:, :], in0=hn_sb[:, :], in1=imeb_sb[:, :], op=ALU.add)
                    dve.drain()
                # strips chunks 2-3 -> s_sb
                if cp_split:
                    dve.wait_ge(PEA, t + 1)
                    nc.vector.tensor_copy(out=s_sb[:, 256:512], in_=ps1[:, 256:512]).then_inc(ACV, 1)
                if epi_v2:
                    # h1 = base + ps2 (after ACT relu1 to avoid concurrent PSUM reads)
                    dve.wait_ge(PEF, t + 1)
                    dve.wait_ge(AI, t + 1)
                    nc.vector.tensor_tensor(
                        out=h1_sb[:, :], in0=base_sb[:, :], in1=ps2[:, :], op=ALU.add,
                    ).then_inc(PSF, 1)
                    # ar1 = alpha * relu1 ; h = h1 + ar1
                    nc.vector.tensor_tensor(
                        out=ar1_sb[:, :], in0=rl_sb[:, :], in1=alpt_sb[:, 0:FREE], op=ALU.mult)
                    dve.drain()
                    nc.vector.tensor_tensor(
                        out=h_sb[:, :], in0=h1_sb[:, :], in1=ar1_sb[:, :], op=ALU.add,
                    ).then_inc(VH, 1)
                    # relu rates (in parallel with ACT sigmoid)
                    if rr_dve:
                        dve.drain()
                        nc.vector.tensor_scalar(
                            out=rr_sb[:, :], in0=h_sb[:, :], scalar1=0.0, scalar2=None,
                            op0=ALU.max)
                        dve.drain()
                else:
                    # v1: ime = (1-a)*ime + a*relu(ps2) per chunk; u = hn+ps2; h = u+ime
                    for c in range(CCH):
                        dve.wait_ge(AI, CCH * t + c + 1)
                        nc.vector.scalar_tensor_tensor(
                            out=ime_sb[:, B * c:B * (c + 1)],
                            in0=ime_sb[:, B * c:B * (c + 1)],
                            scalar=alpt_sb[:, FREE + B * c:FREE + B * c + 1],
                            in1=rl_sb[:, B * c:B * (c + 1)],
                            op0=ALU.mult, op1=ALU.add,
                        )
                    dve.drain()
                    dve.wait_ge(PEF, t + 1)
                    nc.vector.tensor_tensor(
                        out=h1_sb[:, :], in0=hn_sb[:, :], in1=ps2[:, :], op=ALU.add,
                    ).then_inc(PSF, 1)
                    dve.drain()
                    nc.vector.tensor_tensor(
                        out=h_sb[:, :], in0=h1_sb[:, :], in1=ime_sb[:, :], op=ALU.add,
                    ).then_inc(VH, 1)
                # r_{t+1} = select(dmask, sig, relu); r_sb free: ship + readout done
                dve.wait_ge(AR, t + 2)
                if rd_own:
                    dve.wait_ge(PEO, t + 1)
                dve.wait_ge(DO, 32 * (t + 1))
                nc.vector.select(r_sb[:, :], dmask_sb[:, :], rs_sb[:, :], rr_sb[:, :],
                                 add_drain=True).then_inc(VR, 1)
                if epi_v2:
                    # ime' = imeb + ar1 (off-path)
                    nc.vector.tensor_tensor(
                        out=ime_sb[:, :], in0=imeb_sb[:, :], in1=ar1_sb[:, :], op=ALU.add)
                    dve.drain()
                # readout bias add
                dve.wait_ge(PEO, t + 1)
                nc.vector.tensor_scalar(
                    out=o_sb[:, B * t:B * (t + 1)], in0=ps3[:, :],
                    scalar1=bout_sb[:, 0:1], scalar2=None, op0=ALU.add,
                ).then_inc(VO, 1)

    return nc


# ---------------- host-side prep ----------------

def _to_bf16(a):
    return np.asarray(a, np.float32).astype(BF16)


_HAS_BIAS = [False]


def prep_inputs(x, noise, w_rec, w_in, b, d2s, w_out, b_out, mask, T=T_FULL):
    x = np.asarray(x, np.float32)[:T]
    noise = np.asarray(noise, np.float32)[:T]
    w_rec = np.asarray(w_rec, np.float32)
    w_in = np.asarray(w_in, np.float32)
    b = np.asarray(b, np.float32)
    d2s = np.asarray(d2s, np.float32)
    w_out = np.asarray(w_out, np.float32)
    b_out = np.asarray(b_out, np.float32)
    mask = np.asarray(mask, np.float32)
    has_bias = bool(np.any(b != 0.0))
    _HAS_BIAS[0] = has_bias

    # effective recurrent weights with dend->soma coupling folded in, DECAY-scaled
    W = np.zeros((NP_, NP_), np.float32)
    W[:N, :N] = np.abs(w_rec) * mask
    d2s_sr = d2s[:SIZES[1]].reshape(N_BR, SIZES[0])
    d2s_pfc = d2s[SIZES[1]:].reshape(N_BR, SIZES[5])
    for k in range(N_BR):
        W[np.arange(OFF[1] + k * SIZES[0], OFF[1] + (k + 1) * SIZES[0]),
          np.arange(OFF[0], OFF[1])] += d2s_sr[k]
        W[np.arange(OFF[6] + k * SIZES[5], OFF[6] + (k + 1) * SIZES[5]),
          np.arange(OFF[5], OFF[6])] += d2s_pfc[k]
    W *= DECAY
    Wb = _to_bf16(W)

    win_full = np.zeros((N_IN, NP_), np.float32)
    win_full[:, :N] = w_in * DECAY
    winb = _to_bf16(win_full)

    alpha = np.zeros(NP_, np.float32)
    alpha[OFF[6]:OFF[7]] = ALPHA_ME
    dend = np.zeros(NP_, np.float32)
    dend[OFF[1]:OFF[2]] = 1.0
    dend[OFF[6]:OFF[7]] = 1.0

    ns = np.float32(np.float32(np.sqrt(2.0 * DECAY)) * np.float32(NOISE_STD))
    bpad = np.pad(b, (0, NP_ - N))
    noise_p = np.zeros((T, B, NP_), np.float32)
    noise_p[:, :, :N] = ns * noise
    if not has_bias:
        pass  # b == 0: nothing to fold
    else:
        # h-path bias rides the mGluR bias matmul through ps2, NOT noise'
        pass

    xt = np.transpose(x, (2, 0, 1)).reshape(N_IN, T * B)
    xtb = _to_bf16(xt)

    F = np.zeros((128, B), np.float32)
    for j in range(4):
        F[32 * j + np.arange(B), np.arange(B)] = 1.0
    Fb = _to_bf16(F)

    woutb = _to_bf16(w_out.reshape(CCH, 128, N_OUT))   # [4, 128, 3]
    wout_zero = np.zeros_like(woutb)

    in_maps = []
    for core in range(N_CORES):
        cols = slice(SHARD * core, SHARD * (core + 1))
        wshard = np.ascontiguousarray(
            Wb[:, cols].reshape(NCH, 128, SHARD).transpose(1, 0, 2).reshape(128, NCH * SHARD))
        winshard = np.ascontiguousarray(winb[:, cols])
        nshard = noise_p[:, :, cols].reshape(T, B, CCH, 128)
        nshard = np.ascontiguousarray(nshard.transpose(3, 0, 2, 1).reshape(128, T * CCH * B))
        a_sh = alpha[cols].reshape(CCH, 128).T        # [128, 4]
        a_rep = np.repeat(a_sh[:, :, None], B, axis=2).reshape(128, FREE)
        alpt = np.concatenate([a_rep, 1.0 - a_rep], axis=1).astype(np.float32)
        dm = np.repeat(dend[cols].reshape(CCH, 128).T[:, :, None], B, axis=2).reshape(128, FREE)
        wo = woutb if core == 0 else wout_zero
        m = {
            "w": wshard,
            "win": winshard,
            "xt": xtb,
            "noise": nshard,
            "wout": np.ascontiguousarray(wo.transpose(1, 0, 2).reshape(128, CCH * N_OUT)),
            "alpt": alpt,
            "dmask": np.ascontiguousarray(dm.astype(np.uint8)),
            "fmat": Fb,
            "bout": b_out.reshape(N_OUT, 1).astype(np.float32),
        }
        if has_bias:
            bb = np.zeros(SHARD + B, np.float32)
            bb[:SHARD] = DECAY * bpad[cols]
            bb[SHARD:] = 1.0
            m["bb"] = _to_bf16(bb.reshape(1, SHARD + B))
        in_maps.append(m)
    return in_maps


def unshard(out_core0, T=T_FULL):
    o = np.asarray(out_core0, np.float32).reshape(N_OUT, T, B)
    return np.ascontiguousarray(o.transpose(1, 2, 0))


# ---------------- runner (inline; kernel must be self-contained) ----------------

_CACHE = {}


def _get_runner(T=T_FULL):
    key = (T, _HAS_BIAS[0])
    if key in _CACHE:
        return _CACHE[key]
    import jax
    from jax.sharding import Mesh, PartitionSpec, NamedSharding
    from jax.experimental.shard_map import shard_map
    from concourse.bass2jax import _bass_exec_p, install_neuronx_cc_hook, partition_id_tensor

    install_neuronx_cc_hook()
    nc = build_kernel(T, has_bias=_HAS_BIAS[0])

    partition_name = nc.partition_id_tensor.name if nc.partition_id_tensor else None
    in_names, out_names, out_avals, zero_outs = [], [], [], []
    for alloc in nc.m.functions[0].allocations:
        if not isinstance(alloc, mybir.MemoryLocationSet):
            continue
        name = alloc.memorylocations[0].name
        if alloc.kind == "ExternalInput":
            if name != partition_name and (nc.dbg_addr is None or name != nc.dbg_addr.name):
                in_names.append(name)
        elif alloc.kind == "ExternalOutput":
            out_names.append(name)
            shape = tuple(alloc.tensor_shape)
            dtype = mybir.dt.np(alloc.dtype)
            out_avals.append(jax.core.ShapedArray(shape, dtype))
            zero_outs.append(np.zeros(shape, dtype))
    n_params = len(in_names)
    all_in_names = list(in_names) + list(out_names)
    has_dbg = nc.dbg_addr is not None
    if has_dbg:
        all_in_names.append(nc.dbg_addr.name)
    if partition_name is not None:
        all_in_names.append(partition_name)

    def _body(*args):
        operands = list(args)
        if has_dbg:
            operands.append(jax.numpy.zeros((1, 2), jax.numpy.uint32))
        if partition_name is not None:
            operands.append(partition_id_tensor())
        return tuple(_bass_exec_p.bind(
            *operands,
            out_avals=tuple(out_avals),
            in_names=tuple(all_in_names),
            out_names=tuple(out_names),
            lowering_input_output_aliases=(),
            sim_require_finite=True,
            sim_require_nnan=True,
            nc=nc,
        ))

    devices = jax.devices()[:N_CORES]
    mesh = Mesh(np.asarray(devices), ("core",))
    n_outs = len(out_names)
    sharded = jax.jit(
        shard_map(_body, mesh=mesh,
                  in_specs=(PartitionSpec("core"),) * (n_params + n_outs),
                  out_specs=(PartitionSpec("core"),) * n_outs,
                  check_rep=False),
        keep_unused=True,
    )
    sharding = NamedSharding(mesh, PartitionSpec("core"))
    state = dict(nc=nc, in_names=in_names, out_names=out_names, out_avals=out_avals,
                 zero_outs=zero_outs, sharded=sharded, sharding=sharding, mesh=mesh)
    _CACHE[key] = state
    return state


def run_device(in_maps, T=T_FULL, stage=None):
    import jax
    st = _get_runner(T)
    sharding = st["sharding"]
    concat_in = [
        jax.device_put(np.concatenate([np.asarray(m[name]) for m in in_maps], axis=0), sharding)
        for name in st["in_names"]
    ]
    concat_zeros = [
        jax.device_put(np.zeros((N_CORES * z.shape[0], *z.shape[1:]), z.dtype), sharding)
        for z in st["zero_outs"]
    ]
    out_arrs = st["sharded"](*concat_in, *concat_zeros)
    jax.block_until_ready(out_arrs)
    i = st["out_names"].index("out")
    full = np.asarray(out_arrs[i])
    per_core_rows = st["out_avals"][i].shape[0]
    return full[:per_core_rows]


def kernel(**inputs):
    in_maps = prep_inputs(**inputs)
    out0 = run_device(in_maps, T=T_FULL)
    return unshard(out0, T=T_FULL)


if __name__ == "__main__":
    nc = build_kernel(4)
    print("build OK")


# revision 3
# speedup vs baseline: 1.3880x; 1.3880x over previous
"""BioRNN Trainium2 kernel v2: 8-core tensor-parallel recurrence.

Per step: AllGather bf16 rates -> 32 col-tiled matmuls (4 PSUM strips, one
bank) -> fold-transpose on PE -> lean epilogue -> ship rates. vs v1:
  - readout computed from the core's OWN rates during the AllGather window
    (w_out zeroed on cores != 0), off the post-gather critical path
  - strip copies: one N=256 ACT copy + one N=256 DVE copy in parallel;
    fold matmuls ordered to chase copy completion (c2,c3,c0,c1)
  - epilogue on-path is 5 full-width ops via
        ime' = ime*(1-alpha) + alpha*relu(pre)
        h'   = (base + ps2) + alpha*relu1,  base = 0.8h + noise' + ime*(1-a)
    with alpha/(1-alpha) materialized as [128,128] tensors (per-partition
    scalars cannot express per-chunk coefficients), base precomputed during
    the AllGather window; relu rates on DVE in parallel with sigmoid on ACT.
"""
import sys
sys.path.insert(0, '/opt/trn_rl_repo')
import numpy as np

import concourse.bass as bass
import concourse.mybir as mybir

try:
    import ml_dtypes
    BF16 = ml_dtypes.bfloat16
except ImportError:  # pragma: no cover
    import jax.numpy as jnp
    BF16 = jnp.bfloat16

# ---- model constants (hardcoded from the problem spec) ----
SIZES = [512, 1024, 128, 128, 128, 512, 1024, 128, 128, 128]
OFF = np.cumsum([0] + SIZES)
N = int(OFF[-1])            # 3840
NP_ = 4096                  # padded
N_BR = 2
N_IN, N_OUT = 128, 3
T_FULL, B = 100, 32
DECAY = np.float32(10.0 / 50.0)
NOISE_STD = 0.01
N_CORES = 8
SHARD = NP_ // N_CORES      # 512 neurons per core
NCH = NP_ // 128            # 32 k-chunks
CCH = SHARD // 128          # 4 chunks per core
FREE = CCH * B              # 128 free cols of state tiles

_tau_me = np.tile(np.logspace(np.log10(100.0), np.log10(5000.0), SIZES[6] // N_BR), N_BR)
ALPHA_ME = (10.0 / _tau_me).astype(np.float32)

DT32 = mybir.dt.float32
DTBF = mybir.dt.bfloat16
AF = mybir.ActivationFunctionType
ALU = mybir.AluOpType


def build_kernel(T=T_FULL, has_bias=False):
    import os
    variant = os.environ.get("BIO_VARIANT", "")
    rd_own = os.environ.get("BIO_RD", "own") == "own"
    cp_split = os.environ.get("BIO_CP", "act") == "split"
    epi_v2 = os.environ.get("BIO_EPI", "v2") == "v2"
    n_dummy = int(os.environ.get("BIO_DUMMY", "0"))
    rr_dve = os.environ.get("BIO_RR", "dve") == "dve"
    nc = bass.Bass("TRN2", num_devices=N_CORES)

    # ---- DRAM parameters (per-core shards prepped on host) ----
    w_d = nc.declare_dram_parameter("w", [128, NCH * SHARD], DTBF, isOutput=False)
    win_d = nc.declare_dram_parameter("win", [128, SHARD], DTBF, isOutput=False)
    xt_d = nc.declare_dram_parameter("xt", [128, T * B], DTBF, isOutput=False)
    noise_d = nc.declare_dram_parameter("noise", [128, T * FREE], DT32, isOutput=False)
    wout_d = nc.declare_dram_parameter("wout", [128, CCH * N_OUT], DTBF, isOutput=False)
    alpt_d = nc.declare_dram_parameter("alpt", [128, 2 * FREE], DT32, isOutput=False)
    dmask_d = nc.declare_dram_parameter("dmask", [128, FREE], mybir.dt.uint8, isOutput=False)
    fmat_d = nc.declare_dram_parameter("fmat", [128, B], DTBF, isOutput=False)
    bout_d = nc.declare_dram_parameter("bout", [N_OUT, 1], DT32, isOutput=False)
    if has_bias:
        bb_d = nc.declare_dram_parameter("bb", [1, SHARD + B], DTBF, isOutput=False)
    out_d = nc.declare_dram_parameter("out", [N_OUT, T * B], DT32, isOutput=True)

    # ---- collective bounce buffers ----
    in_b = [nc.dram_tensor(f"in_b{p}", [128, FREE], DTBF) for p in range(2)]
    out_b = [nc.dram_tensor(f"out_b{p}", [128 * N_CORES, FREE], DTBF, addr_space="Shared")
             for p in range(2)]

    from contextlib import ExitStack
    with ExitStack() as ctx:
        block = ctx.enter_context(nc.Block())
        sems = {n: ctx.enter_context(nc.semaphore(n)) for n in
                ["DINIT", "DO", "DI", "DI2", "DI3", "DI4", "CC",
                 "PEA", "PEF", "PEO", "ACA", "ACV", "AI", "AR", "PSF",
                 "VH", "VR", "VO", "DO2"]}
        DINIT = sems["DINIT"]; DO = sems["DO"]; CC = sems["CC"]; DO2 = sems["DO2"]
        DI = sems["DI"]; DI2 = sems["DI2"]; DI3 = sems["DI3"]; DI4 = sems["DI4"]
        PEA = sems["PEA"]; PEF = sems["PEF"]; PEO = sems["PEO"]
        ACA = sems["ACA"]; ACV = sems["ACV"]; AI = sems["AI"]; AR = sems["AR"]
        PSF = sems["PSF"]; VH = sems["VH"]; VR = sems["VR"]; VO = sems["VO"]

        def sb(name, shape, dt):
            return ctx.enter_context(nc.sbuf_tensor(name, shape, dt))

        w_sb = sb("w_sb", [128, NCH * SHARD], DTBF)
        win_sb = sb("win_sb", [128, SHARD], DTBF)
        xt_sb = sb("xt_sb", [128, T * B], DTBF)
        noise_sb = sb("noise_sb", [128, T * FREE], DT32)
        wout_sb = sb("wout_sb", [128, CCH * N_OUT], DTBF)
        alpt_sb = sb("alpt_sb", [128, 2 * FREE], DT32)
        dmask_sb = sb("dmask_sb", [128, FREE], mybir.dt.uint8)
        fmat_sb = sb("fmat_sb", [128, B], DTBF)
        bout_sb = sb("bout_sb", [N_OUT, 1], DT32)
        if has_bias:
            bb_sb = sb("bb_sb", [1, SHARD + B], DTBF)
        g_sb = sb("g_sb", [128, N_CORES * FREE], DTBF)
        s_sb = sb("s_sb", [128, SHARD], DTBF)
        h_sb = sb("h_sb", [128, FREE], DT32)
        hn_sb = sb("hn_sb", [128, FREE], DT32)
        imeb_sb = sb("imeb_sb", [128, FREE], DT32)
        base_sb = sb("base_sb", [128, FREE], DT32)
        h1_sb = sb("h1_sb", [128, FREE], DT32)
        ar1_sb = sb("ar1_sb", [128, FREE], DT32)
        ime_sb = sb("ime_sb", [128, FREE], DT32)
        rs_sb = sb("rs_sb", [128, FREE], DT32)
        rl_sb = sb("rl_sb", [128, FREE], DT32)
        rr_sb = sb("rr_sb", [128, FREE], DT32)
        r_sb = sb("r_sb", [128, FREE], DTBF)
        o_sb = sb("o_sb", [N_OUT, T * B], DT32)
        ps1 = ctx.enter_context(nc.psum_tensor("ps1", [128, SHARD], DT32))
        ps2 = ctx.enter_context(nc.psum_tensor("ps2", [128, FREE], DT32))
        ps3 = ctx.enter_context(nc.psum_tensor("ps3", [N_OUT, B], DT32))
        psD = (ctx.enter_context(nc.psum_tensor("psD", [128, SHARD], DT32))
               if n_dummy else None)

        init_pairs = [
            (w_sb, w_d), (win_sb, win_d), (xt_sb, xt_d), (noise_sb, noise_d),
            (wout_sb, wout_d), (alpt_sb, alpt_d), (dmask_sb, dmask_d),
            (fmat_sb, fmat_d), (bout_sb, bout_d),
        ]
        if has_bias:
            init_pairs.append((bb_sb, bb_d))
        N_INIT_DMA = len(init_pairs)

        @block.sync
        def _(sync):
            for dst, src in init_pairs:
                sync.dma_start(out=dst[:, :], in_=src[:, :]).then_inc(DINIT, 16)
            for t in range(T):
                p = t % 2
                # ship r_t lower half
                sync.dma_start(out=in_b[p][0:64, :], in_=r_sb[0:64, :]).wait_op(
                    VR, t + 1, "sem-ge").then_inc(DO, 16)
                if not rd_own and t > 0:
                    sync.wait_ge(PEO, t)
                # gather-in quarters 0-1 (ranks 0-3)
                for q, sem in [(0, DI), (1, DI2)]:
                    ob = out_b[p][256 * q:256 * (q + 1), :].rearrange("(c p) n -> p c n", p=128)
                    gb = g_sb[:, 256 * q:256 * (q + 1)].rearrange("p (c n) -> p c n", c=2)
                    sync.dma_start(out=gb, in_=ob).wait_op(CC, t + 1, "sem-ge").then_inc(sem, 16)
            sync.wait_ge(VO, T)
            sync.dma_start(out=out_d[:, :], in_=o_sb[:, :]).then_inc(DO2, 16)

        @block.gpsimd
        def _(gpsimd):
            for t in range(T):
                p = t % 2
                if variant == "noag":
                    gpsimd.wait_ge(DO, 32 * (t + 1))
                    gpsimd.sem_inc(CC, 1)
                else:
                    gpsimd.collective_compute(
                        "AllGather",
                        ALU.bypass,
                        replica_groups=[list(range(N_CORES))],
                        ins=[in_b[p].ap().opt()],
                        outs=[out_b[p].ap().opt()],
                    ).wait_op(DO, 32 * (t + 1), "sem-ge").then_inc(CC)

        @block.tensor
        def _(pe):
            pe.wait_ge(DINIT, 16 * N_INIT_DMA)
            for t in range(T):
                # ps1 free after copies of t-1
                if t > 0:
                    pe.wait_ge(ACA, t)
                    pe.wait_ge(ACV, t)
                # x_t into strip 0 (runs during the AllGather)
                nc.tensor.matmul(
                    out=ps1[0:32, :],
                    lhsT=xt_sb[:, B * t:B * (t + 1)],
                    rhs=win_sb[:, :],
                    start=True, stop=False,
                    tile_position=(0, 0),
                    skip_group_check=True,
                )
                # readout from own rates r_t (during the AllGather)
                if rd_own:
                    pe.wait_ge(VR, t + 1)
                    if t > 0:
                        pe.wait_ge(VO, t)  # ps3 free
                    for c in range(CCH):
                        mm = nc.tensor.matmul(
                            out=ps3[:, :],
                            lhsT=wout_sb[:, N_OUT * c:N_OUT * (c + 1)],
                            rhs=r_sb[:, B * c:B * (c + 1)],
                            start=(c == 0), stop=(c == CCH - 1),
                            skip_group_check=True,
                        )
                    mm.then_inc(PEO, 1)
                # dummy matmuls to keep PE busy through the AllGather window
                for dmy in range(n_dummy):
                    nc.tensor.matmul(
                        out=psD[0:32, :],
                        lhsT=xt_sb[:, 0:32],
                        rhs=w_sb[:, 0:SHARD],
                        start=True, stop=True,
                        skip_group_check=True,
                    )
                # main recurrent matmuls: 8 groups x 4 col-tiled strips
                pe.wait_ge(DI, 16 * (t + 1))
                waited = {0}
                for g in range(0 if variant == "nomm" else 8):
                    q = g // 2
                    if q not in waited:
                        pe.wait_ge([DI, DI2, DI3, DI4][q], 16 * (t + 1))
                        waited.add(q)
                    for j in range(4):
                        kc = 4 * g + j
                        mm = nc.tensor.matmul(
                            out=ps1[32 * j:32 * (j + 1), :],
                            lhsT=g_sb[:, 32 * kc:32 * (kc + 1)],
                            rhs=w_sb[:, SHARD * kc:SHARD * (kc + 1)],
                            start=(g == 0 and j != 0),
                            stop=(g == 7),
                            skip_group_check=True,
                            tile_position=(0, 32 * j),
                        )
                if variant == "nomm":
                    mm = nc.tensor.matmul(
                        out=ps1[0:32, 0:32], lhsT=xt_sb[:, 0:32],
                        rhs=win_sb[:, 0:32], start=False, stop=True,
                        skip_group_check=True)
                mm.then_inc(PEA, 1)
                # ps2 group: optional mGluR bias, then fold-transpose chasing copies
                if t > 0:
                    pe.wait_ge(AI, (t if epi_v2 else CCH * t))
                    pe.wait_ge(PSF, t)
                if has_bias:
                    for c in range(CCH):
                        nc.tensor.matmul(
                            out=ps2[:, B * c:B * (c + 1)],
                            lhsT=bb_sb[0:1, 128 * c:128 * (c + 1)],
                            rhs=bb_sb[0:1, SHARD:SHARD + B],
                            start=(c == 0), stop=False,
                            skip_group_check=True,
                        )
                for i, c in enumerate((2, 3, 0, 1)):
                    pe.wait_ge(ACV if c >= 2 else ACA, t + 1)
                    mm = nc.tensor.matmul(
                        out=ps2[:, B * c:B * (c + 1)],
                        lhsT=s_sb[:, 128 * c:128 * (c + 1)],
                        rhs=fmat_sb[:, :],
                        start=(i == 0 and not has_bias), stop=(i == 3),
                        skip_group_check=True,
                    )
                mm.then_inc(PEF, 1)
                if not rd_own:
                    if t > 0:
                        pe.wait_ge(VO, t)  # ps3 free
                    for c in range(CCH):
                        mm = nc.tensor.matmul(
                            out=ps3[:, :],
                            lhsT=wout_sb[:, N_OUT * c:N_OUT * (c + 1)],
                            rhs=g_sb[:, B * c:B * (c + 1)],
                            start=(c == 0), stop=(c == CCH - 1),
                            skip_group_check=True,
                        )
                    mm.then_inc(PEO, 1)

        @block.scalar
        def _(act):
            act.wait_ge(DINIT, 16 * N_INIT_DMA)
            # r_0 sigmoid half from h_0 = 0
            act.wait_ge(VH, 1)
            nc.scalar.activation(rs_sb[:, :], h_sb[:, :], AF.Sigmoid).then_inc(AR, 1)
            for t in range(T):
                p = t % 2
                # ship r_t upper half + gather-in quarters 2-3 (DMA queue ops)
                act.dma_start(out=in_b[p][64:128, :], in_=r_sb[64:128, :]).wait_op(
                    VR, t + 1, "sem-ge").then_inc(DO, 16)
                for q, sem in [(2, DI3), (3, DI4)]:
                    ob2 = out_b[p][256 * q:256 * (q + 1), :].rearrange("(c p) n -> p c n", p=128)
                    gb2 = g_sb[:, 256 * q:256 * (q + 1)].rearrange("p (c n) -> p c n", c=2)
                    act.dma_start(out=gb2, in_=ob2).wait_op(CC, t + 1, "sem-ge").then_inc(sem, 16)
                # strips chunks 0-1 -> s_sb (one N=256 copy)
                act.wait_ge(PEA, t + 1)
                if not cp_split:
                    nc.scalar.copy(out=s_sb[:, 256:512], in_=ps1[:, 256:512]).then_inc(ACV, 1)
                nc.scalar.copy(out=s_sb[:, 0:256], in_=ps1[:, 0:256]).then_inc(ACA, 1)
                if epi_v2:
                    # relu1 = relu(ps2)  (= DECAY * relu(pre))
                    act.wait_ge(PEF, t + 1)
                    nc.scalar.activation(rl_sb[:, :], ps2[:, :], AF.Relu).then_inc(AI, 1)
                    # sigmoid rates from h_{t+1}
                    act.wait_ge(VH, t + 2)
                    if rr_dve:
                        nc.scalar.activation(rs_sb[:, :], h_sb[:, :], AF.Sigmoid).then_inc(AR, 1)
                    else:
                        nc.scalar.activation(rs_sb[:, :], h_sb[:, :], AF.Sigmoid)
                        nc.scalar.activation(rr_sb[:, :], h_sb[:, :], AF.Relu).then_inc(AR, 1)
                else:
                    act.wait_ge(PEF, t + 1)
                    for c in range(CCH):
                        nc.scalar.activation(
                            rl_sb[:, B * c:B * (c + 1)],
                            ps2[:, B * c:B * (c + 1)],
                            AF.Relu,
                            scale=alpt_sb[:, B * c:B * c + 1],
                        ).then_inc(AI, 1)
                    act.wait_ge(VH, t + 2)
                    nc.scalar.activation(rs_sb[:, :], h_sb[:, :], AF.Sigmoid)
                    nc.scalar.activation(rr_sb[:, :], h_sb[:, :], AF.Relu).then_inc(AR, 1)

        @block.vector
        def _(dve):
            dve.wait_ge(DINIT, 16 * N_INIT_DMA)
            dve.memset(h_sb[:, :], 0.0)
            dve.memset(ime_sb[:, :], 0.0)
            dve.memset(rr_sb[:, :], 0.0).then_inc(VH, 1)
            dve.drain()
            dve.wait_ge(AR, 1)
            nc.vector.select(r_sb[:, :], dmask_sb[:, :], rs_sb[:, :], rr_sb[:, :],
                             add_drain=True).then_inc(VR, 1)
            for t in range(T):
                # off-path during the AllGather:
                # hn = 0.8h + noise' ; imeb = ime*(1-alpha) ; base = hn + imeb
                nc.vector.scalar_tensor_tensor(
                    out=hn_sb[:, :], in0=h_sb[:, :], scalar=float(1.0 - DECAY),
                    in1=noise_sb[:, FREE * t:FREE * (t + 1)], op0=ALU.mult, op1=ALU.add,
                )
                if epi_v2:
                    nc.vector.tensor_tensor(
                        out=imeb_sb[:, :], in0=ime_sb[:, :], in1=alpt_sb[:, FREE:2 * FREE],
                        op=ALU.mult)
                    dve.drain()
                    nc.vector.tensor_tensor(
                        out=base_sb[:, :], in0=hn_sb[:, :], in1=imeb_sb[:, :], op=ALU.add)
                    dve.drain()
                # strips chunks 2-3 -> s_sb
                if cp_split:
                    dve.wait_ge(PEA, t + 1)
                    nc.vector.tensor_copy(out=s_sb[:, 256:512], in_=ps1[:, 256:512]).then_inc(ACV, 1)
                if epi_v2:
                    # h1 = base + ps2 (after ACT relu1 to avoid concurrent PSUM reads)
                    dve.wait_ge(PEF, t + 1)
                    dve.wait_ge(AI, t + 1)
                    nc.vector.tensor_tensor(
                        out=h1_sb[:, :], in0=base_sb[:, :], in1=ps2[:, :], op=ALU.add,
                    ).then_inc(PSF, 1)
                    # ar1 = alpha * relu1 ; h = h1 + ar1
                    nc.vector.tensor_tensor(
                        out=ar1_sb[:, :], in0=rl_sb[:, :], in1=alpt_sb[:, 0:FREE], op=ALU.mult)
                    dve.drain()
                    nc.vector.tensor_tensor(
                        out=h_sb[:, :], in0=h1_sb[:, :], in1=ar1_sb[:, :], op=ALU.add,
                    ).then_inc(VH, 1)
                    # relu rates (in parallel with ACT sigmoid)
                    if rr_dve:
                        dve.drain()
                        nc.vector.tensor_scalar(
                            out=rr_sb[:, :], in0=h_sb[:, :], scalar1=0.0, scalar2=None,
                            op0=ALU.max)
                        dve.drain()
                else:
                    # v1: ime = (1-a)*ime + a*relu(ps2) per chunk; u = hn+ps2; h = u+ime
                    for c in range(CCH):
                        dve.wait_ge(AI, CCH * t + c + 1)
                        nc.vector.scalar_tensor_tensor(
                            out=ime_sb[:, B * c:B * (c + 1)],
                            in0=ime_sb[:, B * c:B * (c + 1)],
                            scalar=alpt_sb[:, FREE + B * c:FREE + B * c + 1],
                            in1=rl_sb[:, B * c:B * (c + 1)],
                            op0=ALU.mult, op1=ALU.add,
                        )
                    dve.drain()
                    dve.wait_ge(PEF, t + 1)
                    nc.vector.tensor_tensor(
                        out=h1_sb[:, :], in0=hn_sb[:, :], in1=ps2[:, :], op=ALU.add,
                    ).then_inc(PSF, 1)
                    dve.drain()
                    nc.vector.tensor_tensor(
                        out=h_sb[:, :], in0=h1_sb[:, :], in1=ime_sb[:, :], op=ALU.add,
                    ).then_inc(VH, 1)
                # r_{t+1} = select(dmask, sig, relu); r_sb free: ship + readout done
                dve.wait_ge(AR, t + 2)
                if rd_own:
                    dve.wait_ge(PEO, t + 1)
                dve.wait_ge(DO, 32 * (t + 1))
                nc.vector.select(r_sb[:, :], dmask_sb[:, :], rs_sb[:, :], rr_sb[:, :],
                                 add_drain=True).then_inc(VR, 1)
                if epi_v2:
                    # ime' = imeb + ar1 (off-path)
                    nc.vector.tensor_tensor(
                        out=ime_sb[:, :], in0=imeb_sb[:, :], in1=ar1_sb[:, :], op=ALU.add)
                    dve.drain()
                # readout bias add
                dve.wait_ge(PEO, t + 1)
                nc.vector.tensor_scalar(
                    out=o_sb[:, B * t:B * (t + 1)], in0=ps3[:, :],
                    scalar1=bout_sb[:, 0:1], scalar2=None, op0=ALU.add,
                ).then_inc(VO, 1)

    return nc


# ---------------- host-side prep ----------------

def _to_bf16(a):
    return np.asarray(a, np.float32).astype(BF16)


_HAS_BIAS = [False]


def prep_inputs(x, noise, w_rec, w_in, b, d2s, w_out, b_out, mask, T=T_FULL):
    x = np.asarray(x, np.float32)[:T]
    noise = np.asarray(noise, np.float32)[:T]
    w_rec = np.asarray(w_rec, np.float32)
    w_in = np.asarray(w_in, np.float32)
    b = np.asarray(b, np.float32)
    d2s = np.asarray(d2s, np.float32)
    w_out = np.asarray(w_out, np.float32)
    b_out = np.asarray(b_out, np.float32)
    mask = np.asarray(mask, np.float32)
    has_bias = bool(np.any(b != 0.0))
    _HAS_BIAS[0] = has_bias

    # effective recurrent weights with dend->soma coupling folded in, DECAY-scaled
    W = np.zeros((NP_, NP_), np.float32)
    W[:N, :N] = np.abs(w_rec) * mask
    d2s_sr = d2s[:SIZES[1]].reshape(N_BR, SIZES[0])
    d2s_pfc = d2s[SIZES[1]:].reshape(N_BR, SIZES[5])
    for k in range(N_BR):
        W[np.arange(OFF[1] + k * SIZES[0], OFF[1] + (k + 1) * SIZES[0]),
          np.arange(OFF[0], OFF[1])] += d2s_sr[k]
        W[np.arange(OFF[6] + k * SIZES[5], OFF[6] + (k + 1) * SIZES[5]),
          np.arange(OFF[5], OFF[6])] += d2s_pfc[k]
    W *= DECAY
    Wb = _to_bf16(W)

    win_full = np.zeros((N_IN, NP_), np.float32)
    win_full[:, :N] = w_in * DECAY
    winb = _to_bf16(win_full)

    alpha = np.zeros(NP_, np.float32)
    alpha[OFF[6]:OFF[7]] = ALPHA_ME
    dend = np.zeros(NP_, np.float32)
    dend[OFF[1]:OFF[2]] = 1.0
    dend[OFF[6]:OFF[7]] = 1.0

    ns = np.float32(np.float32(np.sqrt(2.0 * DECAY)) * np.float32(NOISE_STD))
    bpad = np.pad(b, (0, NP_ - N))
    noise_p = np.zeros((T, B, NP_), np.float32)
    noise_p[:, :, :N] = ns * noise
    if not has_bias:
        pass  # b == 0: nothing to fold
    else:
        # h-path bias rides the mGluR bias matmul through ps2, NOT noise'
        pass

    xt = np.transpose(x, (2, 0, 1)).reshape(N_IN, T * B)
    xtb = _to_bf16(xt)

    F = np.zeros((128, B), np.float32)
    for j in range(4):
        F[32 * j + np.arange(B), np.arange(B)] = 1.0
    Fb = _to_bf16(F)

    woutb = _to_bf16(w_out.reshape(CCH, 128, N_OUT))   # [4, 128, 3]
    wout_zero = np.zeros_like(woutb)

    in_maps = []
    for core in range(N_CORES):
        cols = slice(SHARD * core, SHARD * (core + 1))
        wshard = np.ascontiguousarray(
            Wb[:, cols].reshape(NCH, 128, SHARD).transpose(1, 0, 2).reshape(128, NCH * SHARD))
        winshard = np.ascontiguousarray(winb[:, cols])
        nshard = noise_p[:, :, cols].reshape(T, B, CCH, 128)
        nshard = np.ascontiguousarray(nshard.transpose(3, 0, 2, 1).reshape(128, T * CCH * B))
        a_sh = alpha[cols].reshape(CCH, 128).T        # [128, 4]
        a_rep = np.repeat(a_sh[:, :, None], B, axis=2).reshape(128, FREE)
        alpt = np.concatenate([a_rep, 1.0 - a_rep], axis=1).astype(np.float32)
        dm = np.repeat(dend[cols].reshape(CCH, 128).T[:, :, None], B, axis=2).reshape(128, FREE)
        wo = woutb if core == 0 else wout_zero
        m = {
            "w": wshard,
            "win": winshard,
            "xt": xtb,
            "noise": nshard,
            "wout": np.ascontiguousarray(wo.transpose(1, 0, 2).reshape(128, CCH * N_OUT)),
            "alpt": alpt,
            "dmask": np.ascontiguousarray(dm.astype(np.uint8)),
            "fmat": Fb,
            "bout": b_out.reshape(N_OUT, 1).astype(np.float32),
        }
        if has_bias:
            bb = np.zeros(SHARD + B, np.float32)
            bb[:SHARD] = DECAY * bpad[cols]
            bb[SHARD:] = 1.0
            m["bb"] = _to_bf16(bb.reshape(1, SHARD + B))
        in_maps.append(m)
    return in_maps


def unshard(out_core0, T=T_FULL):
    o = np.asarray(out_core0, np.float32).reshape(N_OUT, T, B)
    return np.ascontiguousarray(o.transpose(1, 2, 0))


# ---------------- runner (inline; kernel must be self-contained) ----------------

_CACHE = {}


def _get_runner(T=T_FULL):
    key = (T, _HAS_BIAS[0])
    if key in _CACHE:
        return _CACHE[key]
    import jax
    from jax.sharding import Mesh, PartitionSpec, NamedSharding
    from jax.experimental.shard_map import shard_map
    from concourse.bass2jax import _bass_exec_p, install_neuronx_cc_hook, partition_id_tensor

    install_neuronx_cc_hook()
    nc = build_kernel(T, has_bias=_HAS_BIAS[0])

    partition_name = nc.partition_id_tensor.name if nc.partition_id_tensor else None
    in_names, out_names, out_avals, zero_outs = [], [], [], []
    for alloc in nc.m.functions[0].allocations:
        if not isinstance(alloc, mybir.MemoryLocationSet):
            continue
        name = alloc.memorylocations[0].name
        if alloc.kind == "ExternalInput":
            if name != partition_name and (nc.dbg_addr is None or name != nc.dbg_addr.name):
                in_names.append(name)
        elif alloc.kind == "ExternalOutput":
            out_names.append(name)
            shape = tuple(alloc.tensor_shape)
            dtype = mybir.dt.np(alloc.dtype)
            out_avals.append(jax.core.ShapedArray(shape, dtype))
            zero_outs.append(np.zeros(shape, dtype))
    n_params = len(in_names)
    all_in_names = list(in_names) + list(out_names)
    has_dbg = nc.dbg_addr is not None
    if has_dbg:
        all_in_names.append(nc.dbg_addr.name)
    if partition_name is not None:
        all_in_names.append(partition_name)

    def _body(*args):
        operands = list(args)
        if has_dbg:
            operands.append(jax.numpy.zeros((1, 2), jax.numpy.uint32))
        if partition_name is not None:
            operands.append(partition_id_tensor())
        return tuple(_bass_exec_p.bind(
            *operands,
            out_avals=tuple(out_avals),
            in_names=tuple(all_in_names),
            out_names=tuple(out_names),
            lowering_input_output_aliases=(),
            sim_require_finite=True,
            sim_require_nnan=True,
            nc=nc,
        ))

    devices = jax.devices()[:N_CORES]
    mesh = Mesh(np.asarray(devices), ("core",))
    n_outs = len(out_names)
    sharded = jax.jit(
        shard_map(_body, mesh=mesh,
                  in_specs=(PartitionSpec("core"),) * (n_params + n_outs),
                  out_specs=(PartitionSpec("core"),) * n_outs,
                  check_rep=False),
        keep_unused=True,
    )
    sharding = NamedSharding(mesh, PartitionSpec("core"))
    state = dict(nc=nc, in_names=in_names, out_names=out_names, out_avals=out_avals,
                 zero_outs=zero_outs, sharded=sharded, sharding=sharding, mesh=mesh)
    _CACHE[key] = state
    return state


def run_device(in_maps, T=T_FULL, stage=None):
    import jax
    st = _get_runner(T)
    sharding = st["sharding"]
    concat_in = [
        jax.device_put(np.concatenate([np.asarray(m[name]) for m in in_maps], axis=0), sharding)
        for name in st["in_names"]
    ]
    concat_zeros = [
        jax.device_put(np.zeros((N_CORES * z.shape[0], *z.shape[1:]), z.dtype), sharding)
        for z in st["zero_outs"]
    ]
    out_arrs = st["sharded"](*concat_in, *concat_zeros)
    jax.block_until_ready(out_arrs)
    i = st["out_names"].index("out")
    full = np.asarray(out_arrs[i])
    per_core_rows = st["out_avals"][i].shape[0]
    return full[:per_core_rows]


def kernel(**inputs):
    in_maps = prep_inputs(**inputs)
    out0 = run_device(in_maps, T=T_FULL)
    return unshard(out0, T=T_FULL)


if __name__ == "__main__":
    nc = build_kernel(4)
    print("build OK")
